# revision 10
# baseline (speedup 1.0000x reference)
"""Trainium2 Bass kernel for EM matrix-capsule routing (nn_MatrixRouting).

Problem shapes (hardcoded): votes [4, 1152, 1152, 17] f32, beta_v [1,32,1,1],
beta_a [1,32,1], output_dim=32, num_routing=3. Output [4, 32, 6, 6, 17].

Strategy: shard the output-capsule axis C=1152 across 8 cores (144 each).
All EM statistics (sums over the input-capsule axis I) are then core-local,
computed with selector-column matmuls on the TensorEngine (partition-dim
reduction). The only cross-core data needed is the R-normalization row-sum
(shape [B, I] = 18KB), all-reduced per batch with on-chip collectives.

Per EM iteration the kernel streams the 45MB/core votes shard from HBM:
  pass 0          : stats with R0 = const (folded out of mu/sigma).
  pass k (fused)  : phase-1 computes p/ap from iteration k-1 params and the
                    local row-sum partials; AllReduce; phase-2 re-streams the
                    shard to accumulate iteration-k stats with the new R.
Between passes, tiny [6, 384] "small math" computes mu/sigma/a and the
per-(c,p) exp-parameters, which are broadcast across partitions with K=6
selector-row matmuls.
"""

import math
import numpy as np
from contextlib import ExitStack

# ---- problem constants (hardcoded per the task contract) ----
B = 4
I_DIM = 1152
C_DIM = 1152
P_DIM = 16
D_DIM = 17
N_CORES = 8
NUM_ROUTING = 3
O_DIM = 32
WW = 36  # w*w = 6*6 positions per output capsule

EPS = 1e-10
LAMBDA = 1e-4
INV_SQRT_2PI = 1.0 / math.sqrt(2.0 * math.pi)

_NC_CACHE = {}


def _patch_tile_drain():
    """This walrus build only accepts one sync-wait on a CTRL instruction;
    spread the Tile exit-drain waits across single-wait NOPs."""
    import concourse.tile as tile
    import concourse.mybir as mybir
    from concourse.vector_clock import ScopedClock

    if getattr(tile.TileContext, "_drain_patched", False):
        return

    def _patched(self, tick_clock, wait_clock):
        nc = self.nc
        probe = nc.sync.nop()
        wait_clock.add_sem_waits(
            probe.ins, ScopedClock({None: tick_clock.global_clock})
        )
        waits = list(probe.ins.sync_info.on_wait) if probe.ins.sync_info else []
        if probe.ins.sync_info:
            probe.ins.sync_info.on_wait = waits[:1]
        for w in waits[1:]:
            n2 = nc.sync.nop()
            if n2.ins.sync_info is None:
                n2.ins.sync_info = mybir.SyncInfo(on_wait=[w], on_update=[])
            else:
                n2.ins.sync_info.on_wait = [w]
        nc.sync.drain()
        nc.all_engine_barrier()
        assert self.sems is not None
        popped = nc._tile_sem_poison_stack.pop()
        assert popped is self._sem_poison
        nc.clear_and_free_semaphores(list(self.sems.allocated().values()))
        nc.all_engine_barrier()

    tile.TileContext._drain_and_barrier = _patched
    tile.TileContext._drain_patched = True


def build_nc(Bd=B, Id=I_DIM, CL=C_DIM // N_CORES, n_cores=N_CORES,
             num_routing=NUM_ROUTING, Pd=P_DIM):
    """Build the per-core SPMD Bass program (identical on every core)."""
    import concourse.bass as bass
    import concourse.mybir as mybir
    import concourse.tile as tile

    _patch_tile_drain()

    f32 = mybir.dt.float32
    Dd = Pd + 1
    NCH = Id // 128          # i-chunks of 128 partitions
    assert Id % 128 == 0
    CG = 24                  # c's per stats row (CG*Pd = 384 <= 512 psum bank)
    G6 = CL // CG            # stats rows
    assert CL % CG == 0
    RW = CG * Pd             # 384 floats per stats row
    AX = mybir.AxisListType.X
    ALU = mybir.AluOpType
    ACTF = mybir.ActivationFunctionType

    nc = bass.Bass()
    votes = nc.declare_dram_parameter("votes", [Bd, Id, CL, Dd], f32, isOutput=False)
    bv16 = nc.declare_dram_parameter("bv16", [G6, CG], f32, isOutput=False)
    ba_in = nc.declare_dram_parameter("ba", [G6, CG], f32, isOutput=False)
    selc_in = nc.declare_dram_parameter("selc", [128, G6 * G6], f32, isOutput=False)
    selT_in = nc.declare_dram_parameter("selT", [G6, G6 * 128], f32, isOutput=False)
    out = nc.declare_dram_parameter("out", [Bd, CL, Dd], f32, isOutput=True)
    rs_loc = nc.dram_tensor("rs_loc", [Bd, Id], f32)
    rs_sh = nc.dram_tensor("rs_sh", [Bd, Id], f32, addr_space="Shared")

    groups = [list(range(n_cores))]

    with tile.TileContext(nc) as tc, ExitStack() as ctx:
        pconst = ctx.enter_context(tc.tile_pool(name="const", bufs=1))
        pv = ctx.enter_context(tc.tile_pool(name="vt", bufs=3))
        pw = ctx.enter_context(tc.tile_pool(name="work", bufs=1))
        pq = ctx.enter_context(tc.tile_pool(name="qs", bufs=2))
        pap = ctx.enter_context(tc.tile_pool(name="apb", bufs=2))
        prep = ctx.enter_context(tc.tile_pool(name="reps", bufs=1))
        psm = ctx.enter_context(tc.tile_pool(name="small", bufs=1))
        pps = ctx.enter_context(tc.tile_pool(name="psums", bufs=1, space="PSUM"))
        ppb = ctx.enter_context(tc.tile_pool(name="psumb", bufs=2, space="PSUM"))

        # -- constants (host-supplied selector matrices) --
        # selc[:, r, :] = [128, G6] with column r all-ones (stats row select)
        selc = pconst.tile([128, G6, G6], f32)
        nc.sync.dma_start(selc[:].rearrange("p a b -> p (a b)"), selc_in[:])
        # selT[:, r, :] = [G6, 128] with partition-row r all-ones (broadcast)
        selT = pconst.tile([G6, G6, 128], f32)
        nc.sync.dma_start(selT[:].rearrange("p a b -> p (a b)"), selT_in[:])
        bv16_t = pconst.tile([G6, CG], f32)
        nc.sync.dma_start(bv16_t[:], bv16[:])
        ba_t = pconst.tile([G6, CG], f32)
        nc.sync.dma_start(ba_t[:], ba_in[:])
        eps_col = pconst.tile([G6, 1], f32)
        nc.vector.memset(eps_col[:], EPS)

        reps = {}  # (name) -> current rep tiles, rebuilt per (b, k)

        def stats_chunk(j, vt, q_ap, ps0, ps1, ps2):
            """q_ap: [128, CL]. Accumulate q, q*V, q*V^2 sums over i into PSUM."""
            vtV = vt[:, :, 0:Pd]                       # [128, CL, P]
            s1 = pq.tile([128, CL, Pd], f32, tag="s1")
            q_b = q_ap.unsqueeze(2).broadcast_to([128, CL, Pd])
            nc.vector.tensor_tensor(s1[:], vt[:, :, 0:Pd], q_b, op=ALU.mult)
            s2 = pq.tile([128, CL, Pd], f32, tag="s2")
            nc.vector.tensor_tensor(s2[:], s1[:], vtV, op=ALU.mult)
            s1f = s1[:].rearrange("p a b -> p (a b)")
            s2f = s2[:].rearrange("p a b -> p (a b)")
            for r in range(G6):
                st = selc[:, r, :]
                first = j == 0 and r == 0
                last = j == NCH - 1 and r == G6 - 1
                nc.tensor.matmul(ps1[:], st, s1f[:, r * RW : (r + 1) * RW],
                                 start=first, stop=last)
                nc.tensor.matmul(ps2[:], st, s2f[:, r * RW : (r + 1) * RW],
                                 start=first, stop=last)
                nc.tensor.matmul(ps0[:], st, q_ap[:, r * CG : (r + 1) * CG],
                                 start=first, stop=last)

        for b in range(Bd):
            for k in range(num_routing):
                ps0 = pps.tile([G6, CG], f32, tag="ps0")
                ps1 = pps.tile([G6, RW], f32, tag="ps1")
                ps2 = pps.tile([G6, RW], f32, tag="ps2")

                if k == 0:
                    # stats with q = a_ (constant R0 = 1/O folded into sumR)
                    for j in range(NCH):
                        vt = pv.tile([128, CL, Dd], f32, tag="vt")
                        nc.sync.dma_start(vt[:], votes[b, j * 128 : (j + 1) * 128])
                        stats_chunk(j, vt, vt[:, :, Pd], ps0, ps1, ps2)
                else:
                    mu_rep, sa2_rep, e_rep = reps["mu"], reps["sa2"], reps["e"]
                    # ---- phase 1: p / ap / local rowsum ----
                    apb = pap.tile([128, NCH * CL], f32, tag="apb")
                    rs_all = pap.tile([128, NCH], f32, tag="rsall")
                    for j in range(NCH):
                        vt = pv.tile([128, CL, Dd], f32, tag="vt")
                        nc.sync.dma_start(vt[:], votes[b, j * 128 : (j + 1) * 128])
                        d = pw.tile([128, CL, Pd], f32, tag="d")
                        nc.vector.tensor_tensor(d[:], vt[:, :, 0:Pd], mu_rep[:],
                                                op=ALU.subtract)
                        dp = pw.tile([128, CL, Pd], f32, tag="dp")
                        nc.vector.tensor_tensor(dp[:], d[:], sa2_rep[:], op=ALU.mult)
                        u = pw.tile([128, CL, Pd], f32, tag="u")
                        nc.scalar.activation(u[:], dp[:], ACTF.Square)
                        u2 = pw.tile([128, CL, Pd], f32, tag="u2")
                        nc.scalar.activation(u2[:], u[:], ACTF.Exp, scale=-1.0)
                        w = pw.tile([128, CL, Pd], f32, tag="w")
                        nc.vector.tensor_tensor(w[:], u2[:], e_rep[:], op=ALU.mult)
                        ap_sl = apb[:, j * CL : (j + 1) * CL]
                        nc.vector.reduce_sum(ap_sl, w[:], axis=AX)
                        nc.vector.reduce_sum(rs_all[:, j : j + 1], ap_sl, axis=AX)
                    nc.sync.dma_start(
                        rs_loc[b].rearrange("(j p) -> p j", p=128), rs_all[:]
                    )
                    nc.gpsimd.collective_compute(
                        "AllReduce", ALU.add, replica_groups=groups,
                        ins=[rs_loc[b]], outs=[rs_sh[b]],
                    )
                    rsg = pap.tile([128, NCH], f32, tag="rsg")
                    nc.sync.dma_start(
                        rsg[:], rs_sh[b].rearrange("(j p) -> p j", p=128)
                    )
                    nc.vector.tensor_scalar_add(rsg[:], rsg[:], EPS)
                    rcp = pap.tile([128, NCH], f32, tag="rcp")
                    nc.vector.reciprocal(rcp[:], rsg[:])
                    # ---- phase 2: stats with R_k = ap * rcp (times a_) ----
                    for j in range(NCH):
                        vt = pv.tile([128, CL, Dd], f32, tag="vt")
                        nc.sync.dma_start(vt[:], votes[b, j * 128 : (j + 1) * 128])
                        q = pq.tile([128, CL], f32, tag="q")
                        nc.vector.scalar_tensor_tensor(
                            q[:], apb[:, j * CL : (j + 1) * CL],
                            rcp[:, j : j + 1], vt[:, :, Pd],
                            op0=ALU.mult, op1=ALU.mult,
                        )
                        stats_chunk(j, vt, q[:], ps0, ps1, ps2)

                # ---- small math on [G6, RW]: mu, sigma, a, next-pass params --
                sb0 = psm.tile([G6, CG], f32, tag="sb0")
                nc.scalar.copy(sb0[:], ps0[:])
                sb1 = psm.tile([G6, CG, Pd], f32, tag="sb1")
                nc.scalar.copy(sb1[:].rearrange("p a b -> p (a b)"), ps1[:])
                sb2 = psm.tile([G6, CG, Pd], f32, tag="sb2")
                nc.scalar.copy(sb2[:].rearrange("p a b -> p (a b)"), ps2[:])

                rS = psm.tile([G6, CG], f32, tag="rS")
                nc.vector.reciprocal(rS[:], sb0[:])
                rS_b = rS[:].unsqueeze(2).broadcast_to([G6, CG, Pd])
                mu6 = psm.tile([G6, CG, Pd], f32, tag="mu6")
                nc.vector.tensor_tensor(mu6[:], sb1[:], rS_b, op=ALU.mult)
                ex2 = psm.tile([G6, CG, Pd], f32, tag="ex2")
                nc.vector.tensor_tensor(ex2[:], sb2[:], rS_b, op=ALU.mult)
                mu2 = psm.tile([G6, CG, Pd], f32, tag="mu2")
                nc.vector.tensor_tensor(mu2[:], mu6[:], mu6[:], op=ALU.mult)
                sig2 = psm.tile([G6, CG, Pd], f32, tag="sig2")
                nc.vector.tensor_tensor(sig2[:], ex2[:], mu2[:], op=ALU.subtract)
                sigma = psm.tile([G6, CG, Pd], f32, tag="sigma")
                nc.scalar.activation(sigma[:], sig2[:], ACTF.Sqrt)
                logs = psm.tile([G6, CG, Pd], f32, tag="logs")
                nc.scalar.activation(logs[:], sigma[:], ACTF.Ln, bias=eps_col[:])
                sumlog = psm.tile([G6, CG], f32, tag="sumlog")
                nc.vector.reduce_sum(sumlog[:], logs[:], axis=AX)
                cst = psm.tile([G6, CG], f32, tag="cst")
                nc.vector.tensor_tensor(cst[:], sumlog[:], bv16_t[:], op=ALU.add)
                if k == 0:
                    se = psm.tile([G6, CG], f32, tag="se")
                    nc.vector.tensor_scalar_mul(se[:], sb0[:], 1.0 / O_DIM)
                    sum_r_eff = se
                else:
                    sum_r_eff = sb0
                cst2 = psm.tile([G6, CG], f32, tag="cst2")
                nc.vector.tensor_tensor(cst2[:], cst[:], sum_r_eff[:], op=ALU.mult)
                ain = psm.tile([G6, CG], f32, tag="ain")
                nc.vector.tensor_tensor(ain[:], ba_t[:], cst2[:], op=ALU.subtract)
                a_t = psm.tile([G6, CG], f32, tag="a_t")
                nc.scalar.activation(a_t[:], ain[:], ACTF.Sigmoid, scale=LAMBDA)

                if k == num_routing - 1:
                    nc.sync.dma_start(
                        out[b, :, 0:Pd].rearrange("(r c) p -> r c p", r=G6), mu6[:]
                    )
                    nc.sync.dma_start(
                        out[b, :, Pd].rearrange("(r c) -> r c", r=G6), a_t[:]
                    )
                else:
                    # E = a * 1/(sigma+eps) * (2pi)^-1/2 ; sA2 = sqrt(1/(2 sig2))
                    sigeps = psm.tile([G6, CG, Pd], f32, tag="sigeps")
                    nc.vector.tensor_scalar_add(sigeps[:], sigma[:], EPS)
                    rsig = psm.tile([G6, CG, Pd], f32, tag="rsig")
                    nc.vector.reciprocal(rsig[:], sigeps[:])
                    a_b = a_t[:].unsqueeze(2).broadcast_to([G6, CG, Pd])
                    e6 = psm.tile([G6, CG, Pd], f32, tag="e6")
                    nc.vector.tensor_tensor(e6[:], rsig[:], a_b, op=ALU.mult)
                    nc.vector.tensor_scalar_mul(e6[:], e6[:], INV_SQRT_2PI)
                    t2s = psm.tile([G6, CG, Pd], f32, tag="t2s")
                    nc.vector.tensor_scalar_mul(t2s[:], sig2[:], 2.0)
                    rt2 = psm.tile([G6, CG, Pd], f32, tag="rt2")
                    nc.vector.reciprocal(rt2[:], t2s[:])
                    sa26 = psm.tile([G6, CG, Pd], f32, tag="sa26")
                    nc.scalar.activation(sa26[:], rt2[:], ACTF.Sqrt)

                    mu_rep = prep.tile([128, CL, Pd], f32, tag="mu_rep")
                    sa2_rep = prep.tile([128, CL, Pd], f32, tag="sa2_rep")
                    e_rep = prep.tile([128, CL, Pd], f32, tag="e_rep")
                    for (src, rep) in ((mu6, mu_rep), (sa26, sa2_rep), (e6, e_rep)):
                        srcf = src[:].rearrange("p a b -> p (a b)")
                        repf = rep[:].rearrange("p a b -> p (a b)")
                        for r in range(G6):
                            pb = ppb.tile([128, RW], f32, tag="pb")
                            nc.tensor.matmul(pb[:], selT[:, r, :],
                                             srcf[:, :], start=True, stop=True)
                            if r % 2 == 0:
                                nc.scalar.copy(repf[:, r * RW : (r + 1) * RW], pb[:])
                            else:
                                nc.vector.tensor_copy(
                                    repf[:, r * RW : (r + 1) * RW], pb[:])
                    reps = {"mu": mu_rep, "sa2": sa2_rep, "e": e_rep}
    _split_sync_waits(nc)
    return nc


def _split_sync_waits(nc, max_waits=1):
    """This walrus build accepts at most one sync-wait per instruction;
    move excess waits onto preceding same-engine NOPs."""
    import concourse.mybir as mybir

    uid = [0]
    for fn in nc.m.functions:
        for bb in fn.blocks:
            insts = bb.instructions
            out = []
            for inst in insts:
                si = inst.sync_info
                if si is not None and si.on_wait and len(si.on_wait) > max_waits:
                    waits = list(si.on_wait)
                    keep = waits[-max_waits:]
                    for w in waits[:-max_waits]:
                        uid[0] += 1
                        nop = mybir.InstNoOp(
                            name=f"I-waitsplit-{uid[0]}", ins=[], outs=[])
                        nop.engine = inst.engine
                        nop.sync_info = mybir.SyncInfo(on_wait=[w], on_update=[])
                        out.append(nop)
                    si.on_wait = keep
                out.append(inst)
            bb.instructions = out
    return nc


# ------------------------- host-side wrapper ----------------------------

def make_selectors(G6):
    """Host-built selector matrices for stats-row and broadcast matmuls."""
    selc = np.zeros((128, G6, G6), np.float32)
    for r in range(G6):
        selc[:, r, r] = 1.0
    selT = np.zeros((G6, G6, 128), np.float32)
    for r in range(G6):
        selT[r, r, :] = 1.0
    return (np.ascontiguousarray(selc.reshape(128, G6 * G6)),
            np.ascontiguousarray(selT.reshape(G6, G6 * 128)))


def _get_nc():
    key = "full"
    if key not in _NC_CACHE:
        _NC_CACHE[key] = build_nc()
    return _NC_CACHE[key]


def kernel(**inputs) -> np.ndarray:
    from concourse.bass_utils import run_bass_kernel_spmd

    votes = np.ascontiguousarray(np.asarray(inputs["votes"], dtype=np.float32))
    beta_v = np.asarray(inputs["beta_v"], dtype=np.float32).reshape(-1)
    beta_a = np.asarray(inputs["beta_a"], dtype=np.float32).reshape(-1)
    output_dim = int(np.asarray(inputs["output_dim"]))
    num_routing = int(np.asarray(inputs["num_routing"]))
    assert votes.shape == (B, I_DIM, C_DIM, D_DIM), votes.shape
    assert output_dim == O_DIM and num_routing == NUM_ROUTING

    CL = C_DIM // N_CORES
    G6, CG = CL // 24, 24
    bvc = np.repeat(beta_v, WW)   # [C]
    bac = np.repeat(beta_a, WW)
    nc = _get_nc()
    selc_np, selT_np = make_selectors(G6)
    in_maps = []
    for c in range(N_CORES):
        sl = slice(c * CL, (c + 1) * CL)
        in_maps.append({
            "votes": np.ascontiguousarray(votes[:, :, sl, :]),
            "bv16": np.ascontiguousarray(
                (P_DIM * bvc[sl]).reshape(G6, CG).astype(np.float32)),
            "ba": np.ascontiguousarray(bac[sl].reshape(G6, CG).astype(np.float32)),
            "selc": selc_np,
            "selT": selT_np,
        })
    res = run_bass_kernel_spmd(nc, in_maps, list(range(N_CORES)))
    parts = [res.results[i]["out"] for i in range(N_CORES)]
    full = np.concatenate(parts, axis=1)          # [B, C, D]
    w = int(math.sqrt(C_DIM // O_DIM))
    return full.reshape(B, O_DIM, w, w, D_DIM).astype(np.float32)


# revision 13
# speedup vs baseline: 1.3490x; 1.3490x over previous
"""Trainium2 Bass kernel for EM matrix-capsule routing (nn_MatrixRouting).

Problem shapes (hardcoded): votes [4, 1152, 1152, 17] f32, beta_v [1,32,1,1],
beta_a [1,32,1], output_dim=32, num_routing=3. Output [4, 32, 6, 6, 17].

Strategy: shard the output-capsule axis C=1152 across 8 cores (144 each).
All EM statistics (sums over the input-capsule axis I) are then core-local,
computed with selector-column matmuls on the TensorEngine (partition-dim
reduction). The only cross-core data needed is the R-normalization row-sum
(shape [B, I] = 18KB), all-reduced per batch with on-chip collectives.

Per EM iteration the kernel streams the 45MB/core votes shard from HBM:
  pass 0          : stats with R0 = const (folded out of mu/sigma).
  pass k (fused)  : phase-1 computes p/ap from iteration k-1 params and the
                    local row-sum partials; AllReduce; phase-2 re-streams the
                    shard to accumulate iteration-k stats with the new R.
Between passes, tiny [6, 384] "small math" computes mu/sigma/a and the
per-(c,p) exp-parameters, which are broadcast across partitions with K=6
selector-row matmuls.
"""

import math
import numpy as np
from contextlib import ExitStack

# ---- problem constants (hardcoded per the task contract) ----
B = 4
I_DIM = 1152
C_DIM = 1152
P_DIM = 16
D_DIM = 17
N_CORES = 8
NUM_ROUTING = 3
O_DIM = 32
WW = 36  # w*w = 6*6 positions per output capsule

EPS = 1e-10
LAMBDA = 1e-4
INV_SQRT_2PI = 1.0 / math.sqrt(2.0 * math.pi)

_NC_CACHE = {}


def _patch_tile_drain():
    """This walrus build only accepts one sync-wait on a CTRL instruction;
    spread the Tile exit-drain waits across single-wait NOPs."""
    import concourse.tile as tile
    import concourse.mybir as mybir
    from concourse.vector_clock import ScopedClock

    if getattr(tile.TileContext, "_drain_patched", False):
        return

    def _patched(self, tick_clock, wait_clock):
        nc = self.nc
        probe = nc.sync.nop()
        wait_clock.add_sem_waits(
            probe.ins, ScopedClock({None: tick_clock.global_clock})
        )
        waits = list(probe.ins.sync_info.on_wait) if probe.ins.sync_info else []
        if probe.ins.sync_info:
            probe.ins.sync_info.on_wait = waits[:1]
        for w in waits[1:]:
            n2 = nc.sync.nop()
            if n2.ins.sync_info is None:
                n2.ins.sync_info = mybir.SyncInfo(on_wait=[w], on_update=[])
            else:
                n2.ins.sync_info.on_wait = [w]
        nc.sync.drain()
        nc.all_engine_barrier()
        assert self.sems is not None
        popped = nc._tile_sem_poison_stack.pop()
        assert popped is self._sem_poison
        nc.clear_and_free_semaphores(list(self.sems.allocated().values()))
        nc.all_engine_barrier()

    tile.TileContext._drain_and_barrier = _patched
    tile.TileContext._drain_patched = True


def build_nc(Bd=B, Id=I_DIM, CL=C_DIM // N_CORES, n_cores=N_CORES,
             num_routing=NUM_ROUTING, Pd=P_DIM, split_waits=True):
    """Build the per-core SPMD Bass program (identical on every core)."""
    import concourse.bass as bass
    import concourse.mybir as mybir
    import concourse.tile as tile

    _patch_tile_drain()

    f32 = mybir.dt.float32
    bf = mybir.dt.bfloat16
    Dd = Pd + 1
    NCH = Id // 128          # i-chunks of 128 partitions
    assert Id % 128 == 0
    CG = 24                  # c's per stats row (CG*Pd = 384 <= 512 psum bank)
    G6 = CL // CG            # stats rows
    assert CL % CG == 0
    RW = CG * Pd             # 384 floats per stats row
    AX = mybir.AxisListType.X
    ALU = mybir.AluOpType
    ACTF = mybir.ActivationFunctionType

    nc = bass.Bass()
    votes = nc.declare_dram_parameter("votes", [Bd, Id, CL, Dd], f32, isOutput=False)
    bv16 = nc.declare_dram_parameter("bv16", [G6, CG], f32, isOutput=False)
    ba_in = nc.declare_dram_parameter("ba", [G6, CG], f32, isOutput=False)
    selc_in = nc.declare_dram_parameter("selc", [128, G6 * G6], f32, isOutput=False)
    selT_in = nc.declare_dram_parameter("selT", [G6, G6 * 128], f32, isOutput=False)
    out = nc.declare_dram_parameter("out", [Bd, CL, Dd], f32, isOutput=True)
    rs_loc = nc.dram_tensor("rs_loc", [Bd, Id], f32)
    rs_sh = nc.dram_tensor("rs_sh", [Bd, Id], f32, addr_space="Shared")
    votes_bv = nc.dram_tensor("votes_bv", [Bd, Id, CL, Pd], bf)
    votes_ba = nc.dram_tensor("votes_ba", [Bd, Id, CL], bf)

    groups = [list(range(n_cores))]

    with tile.TileContext(nc) as tc, ExitStack() as ctx:
        pconst = ctx.enter_context(tc.tile_pool(name="const", bufs=1))
        pv = ctx.enter_context(tc.tile_pool(name="vt", bufs=3))
        pw = ctx.enter_context(tc.tile_pool(name="work", bufs=1))
        pq = ctx.enter_context(tc.tile_pool(name="qs", bufs=2))
        pap = ctx.enter_context(tc.tile_pool(name="apb", bufs=2))
        prep = ctx.enter_context(tc.tile_pool(name="reps", bufs=1))
        psm = ctx.enter_context(tc.tile_pool(name="small", bufs=1))
        pps = ctx.enter_context(tc.tile_pool(name="psums", bufs=1, space="PSUM"))
        ppb = ctx.enter_context(tc.tile_pool(name="psumb", bufs=2, space="PSUM"))

        # -- constants (host-supplied selector matrices) --
        # selc[:, r, :] = [128, G6] with column r all-ones (stats row select)
        selc = pconst.tile([128, G6, G6], f32)
        nc.sync.dma_start(selc[:].rearrange("p a b -> p (a b)"), selc_in[:])
        # selT[:, r, :] = [G6, 128] with partition-row r all-ones (broadcast)
        selT = pconst.tile([G6, G6, 128], f32)
        nc.sync.dma_start(selT[:].rearrange("p a b -> p (a b)"), selT_in[:])
        bv16_t = pconst.tile([G6, CG], f32)
        nc.sync.dma_start(bv16_t[:], bv16[:])
        ba_t = pconst.tile([G6, CG], f32)
        nc.sync.dma_start(ba_t[:], ba_in[:])
        eps_col = pconst.tile([G6, 1], f32)
        nc.vector.memset(eps_col[:], EPS)
        selcb = pconst.tile([128, G6, G6], bf)
        nc.vector.tensor_copy(selcb[:], selc[:])
        selTb = pconst.tile([G6, G6, 128], bf)
        nc.vector.tensor_copy(selTb[:], selT[:])

        reps = {}  # (name) -> current rep tiles, rebuilt per (b, k)

        def stats_chunk(j, vtv, q_ap, ps0, ps1, ps2):
            """vtv: [128, CL, P] bf16; q_ap: [128, CL] bf16. Accumulate
            q, q*V, q*V^2 sums over i into f32 PSUM."""
            s1 = pq.tile([128, CL, Pd], bf, tag="s1")
            q_b = q_ap.unsqueeze(2).broadcast_to([128, CL, Pd])
            nc.vector.tensor_tensor(s1[:], vtv[:], q_b, op=ALU.mult)
            s2 = pq.tile([128, CL, Pd], bf, tag="s2")
            nc.vector.tensor_tensor(s2[:], s1[:], vtv[:], op=ALU.mult)
            s1f = s1[:].rearrange("p a b -> p (a b)")
            s2f = s2[:].rearrange("p a b -> p (a b)")
            for r in range(G6):
                st = selcb[:, r, :]
                first = j == 0 and r == 0
                last = j == NCH - 1 and r == G6 - 1
                nc.tensor.matmul(ps1[:], st, s1f[:, r * RW : (r + 1) * RW],
                                 start=first, stop=last)
                nc.tensor.matmul(ps2[:], st, s2f[:, r * RW : (r + 1) * RW],
                                 start=first, stop=last)
                nc.tensor.matmul(ps0[:], st, q_ap[:, r * CG : (r + 1) * CG],
                                 start=first, stop=last)

        for b in range(Bd):
            for k in range(num_routing):
                ps0 = pps.tile([G6, CG], f32, tag="ps0")
                ps1 = pps.tile([G6, RW], f32, tag="ps1")
                ps2 = pps.tile([G6, RW], f32, tag="ps2")

                if k == 0:
                    # stats with q = a_ (constant R0 = 1/O folded into sumR);
                    # also persist a bf16 copy of the shard for passes 1..n
                    for j in range(NCH):
                        vt = pv.tile([128, CL, Dd], f32, tag="vt")
                        nc.sync.dma_start(vt[:], votes[b, j * 128 : (j + 1) * 128])
                        vtv = pv.tile([128, CL, Pd], bf, tag="vtv")
                        nc.scalar.copy(vtv[:], vt[:, :, 0:Pd])
                        vta = pv.tile([128, CL], bf, tag="vta")
                        nc.vector.tensor_copy(vta[:], vt[:, :, Pd])
                        nc.sync.dma_start(votes_bv[b, j * 128 : (j + 1) * 128], vtv[:])
                        nc.sync.dma_start(votes_ba[b, j * 128 : (j + 1) * 128], vta[:])
                        stats_chunk(j, vtv, vta[:], ps0, ps1, ps2)
                else:
                    mu_rep, sa2_rep, e_rep = reps["mu"], reps["sa2"], reps["e"]
                    # ---- phase 1: p / ap / local rowsum ----
                    apb = pap.tile([128, NCH * CL], f32, tag="apb")
                    rs_all = pap.tile([128, NCH], f32, tag="rsall")
                    for j in range(NCH):
                        vtv = pv.tile([128, CL, Pd], bf, tag="vtv")
                        nc.sync.dma_start(vtv[:], votes_bv[b, j * 128 : (j + 1) * 128])
                        d = pw.tile([128, CL, Pd], bf, tag="d")
                        nc.vector.tensor_tensor(d[:], vtv[:], mu_rep[:],
                                                op=ALU.subtract)
                        dp = pw.tile([128, CL, Pd], bf, tag="dp")
                        nc.vector.tensor_tensor(dp[:], d[:], sa2_rep[:], op=ALU.mult)
                        u = pw.tile([128, CL, Pd], bf, tag="u")
                        nc.scalar.activation(u[:], dp[:], ACTF.Square)
                        u2 = pw.tile([128, CL, Pd], bf, tag="u2")
                        nc.scalar.activation(u2[:], u[:], ACTF.Exp, scale=-1.0)
                        w = pw.tile([128, CL, Pd], bf, tag="w")
                        nc.vector.tensor_tensor(w[:], u2[:], e_rep[:], op=ALU.mult)
                        ap_sl = apb[:, j * CL : (j + 1) * CL]
                        nc.vector.reduce_sum(ap_sl, w[:], axis=AX)
                        nc.vector.reduce_sum(rs_all[:, j : j + 1], ap_sl, axis=AX)
                    nc.sync.dma_start(
                        rs_loc[b].rearrange("(j p) -> p j", p=128), rs_all[:]
                    )
                    nc.gpsimd.collective_compute(
                        "AllReduce", ALU.add, replica_groups=groups,
                        ins=[rs_loc[b]], outs=[rs_sh[b]],
                    )
                    rsg = pap.tile([128, NCH], f32, tag="rsg")
                    nc.sync.dma_start(
                        rsg[:], rs_sh[b].rearrange("(j p) -> p j", p=128)
                    )
                    nc.vector.tensor_scalar_add(rsg[:], rsg[:], EPS)
                    rcp = pap.tile([128, NCH], f32, tag="rcp")
                    nc.vector.reciprocal(rcp[:], rsg[:])
                    # ---- phase 2: stats with R_k = ap * rcp (times a_) ----
                    for j in range(NCH):
                        vtv = pv.tile([128, CL, Pd], bf, tag="vtv")
                        nc.sync.dma_start(vtv[:], votes_bv[b, j * 128 : (j + 1) * 128])
                        vta = pv.tile([128, CL], bf, tag="vta")
                        nc.sync.dma_start(vta[:], votes_ba[b, j * 128 : (j + 1) * 128])
                        ac = pq.tile([128, CL], f32, tag="ac")
                        nc.vector.tensor_copy(ac[:], vta[:])
                        qf = pq.tile([128, CL], f32, tag="qf")
                        nc.vector.scalar_tensor_tensor(
                            qf[:], apb[:, j * CL : (j + 1) * CL],
                            rcp[:, j : j + 1], ac[:],
                            op0=ALU.mult, op1=ALU.mult,
                        )
                        qb = pq.tile([128, CL], bf, tag="qb")
                        nc.vector.tensor_copy(qb[:], qf[:])
                        stats_chunk(j, vtv, qb[:], ps0, ps1, ps2)

                # ---- small math on [G6, RW]: mu, sigma, a, next-pass params --
                sb0 = psm.tile([G6, CG], f32, tag="sb0")
                nc.scalar.copy(sb0[:], ps0[:])
                sb1 = psm.tile([G6, CG, Pd], f32, tag="sb1")
                nc.scalar.copy(sb1[:].rearrange("p a b -> p (a b)"), ps1[:])
                sb2 = psm.tile([G6, CG, Pd], f32, tag="sb2")
                nc.scalar.copy(sb2[:].rearrange("p a b -> p (a b)"), ps2[:])

                rS = psm.tile([G6, CG], f32, tag="rS")
                nc.vector.reciprocal(rS[:], sb0[:])
                rS_b = rS[:].unsqueeze(2).broadcast_to([G6, CG, Pd])
                mu6 = psm.tile([G6, CG, Pd], f32, tag="mu6")
                nc.vector.tensor_tensor(mu6[:], sb1[:], rS_b, op=ALU.mult)
                ex2 = psm.tile([G6, CG, Pd], f32, tag="ex2")
                nc.vector.tensor_tensor(ex2[:], sb2[:], rS_b, op=ALU.mult)
                mu2 = psm.tile([G6, CG, Pd], f32, tag="mu2")
                nc.vector.tensor_tensor(mu2[:], mu6[:], mu6[:], op=ALU.mult)
                sig2 = psm.tile([G6, CG, Pd], f32, tag="sig2")
                nc.vector.tensor_tensor(sig2[:], ex2[:], mu2[:], op=ALU.subtract)
                sigma = psm.tile([G6, CG, Pd], f32, tag="sigma")
                nc.scalar.activation(sigma[:], sig2[:], ACTF.Sqrt)
                logs = psm.tile([G6, CG, Pd], f32, tag="logs")
                nc.scalar.activation(logs[:], sigma[:], ACTF.Ln, bias=eps_col[:])
                sumlog = psm.tile([G6, CG], f32, tag="sumlog")
                nc.vector.reduce_sum(sumlog[:], logs[:], axis=AX)
                cst = psm.tile([G6, CG], f32, tag="cst")
                nc.vector.tensor_tensor(cst[:], sumlog[:], bv16_t[:], op=ALU.add)
                if k == 0:
                    se = psm.tile([G6, CG], f32, tag="se")
                    nc.vector.tensor_scalar_mul(se[:], sb0[:], 1.0 / O_DIM)
                    sum_r_eff = se
                else:
                    sum_r_eff = sb0
                cst2 = psm.tile([G6, CG], f32, tag="cst2")
                nc.vector.tensor_tensor(cst2[:], cst[:], sum_r_eff[:], op=ALU.mult)
                ain = psm.tile([G6, CG], f32, tag="ain")
                nc.vector.tensor_tensor(ain[:], ba_t[:], cst2[:], op=ALU.subtract)
                a_t = psm.tile([G6, CG], f32, tag="a_t")
                nc.scalar.activation(a_t[:], ain[:], ACTF.Sigmoid, scale=LAMBDA)

                if k == num_routing - 1:
                    nc.sync.dma_start(
                        out[b, :, 0:Pd].rearrange("(r c) p -> r c p", r=G6), mu6[:]
                    )
                    nc.sync.dma_start(
                        out[b, :, Pd].rearrange("(r c) -> r c", r=G6), a_t[:]
                    )
                else:
                    # E = a * 1/(sigma+eps) * (2pi)^-1/2 ; sA2 = sqrt(1/(2 sig2))
                    sigeps = psm.tile([G6, CG, Pd], f32, tag="sigeps")
                    nc.vector.tensor_scalar_add(sigeps[:], sigma[:], EPS)
                    rsig = psm.tile([G6, CG, Pd], f32, tag="rsig")
                    nc.vector.reciprocal(rsig[:], sigeps[:])
                    a_b = a_t[:].unsqueeze(2).broadcast_to([G6, CG, Pd])
                    e6 = psm.tile([G6, CG, Pd], f32, tag="e6")
                    nc.vector.tensor_tensor(e6[:], rsig[:], a_b, op=ALU.mult)
                    nc.vector.tensor_scalar_mul(e6[:], e6[:], INV_SQRT_2PI)
                    t2s = psm.tile([G6, CG, Pd], f32, tag="t2s")
                    nc.vector.tensor_scalar_mul(t2s[:], sig2[:], 2.0)
                    rt2 = psm.tile([G6, CG, Pd], f32, tag="rt2")
                    nc.vector.reciprocal(rt2[:], t2s[:])
                    sa26 = psm.tile([G6, CG, Pd], f32, tag="sa26")
                    nc.scalar.activation(sa26[:], rt2[:], ACTF.Sqrt)

                    mu_rep = prep.tile([128, CL, Pd], bf, tag="mu_rep")
                    sa2_rep = prep.tile([128, CL, Pd], bf, tag="sa2_rep")
                    e_rep = prep.tile([128, CL, Pd], bf, tag="e_rep")
                    for (nm, src, rep) in (("mu", mu6, mu_rep),
                                           ("sa2", sa26, sa2_rep),
                                           ("e", e6, e_rep)):
                        srcb = psm.tile([G6, CG, Pd], bf, tag="srcb_" + nm)
                        nc.scalar.copy(srcb[:], src[:])
                        srcf = srcb[:].rearrange("p a b -> p (a b)")
                        repf = rep[:].rearrange("p a b -> p (a b)")
                        for r in range(G6):
                            pb = ppb.tile([128, RW], f32, tag="pb")
                            nc.tensor.matmul(pb[:], selTb[:, r, :],
                                             srcf[:, :], start=True, stop=True)
                            if r % 2 == 0:
                                nc.scalar.copy(repf[:, r * RW : (r + 1) * RW], pb[:])
                            else:
                                nc.vector.tensor_copy(
                                    repf[:, r * RW : (r + 1) * RW], pb[:])
                    reps = {"mu": mu_rep, "sa2": sa2_rep, "e": e_rep}
    if split_waits:
        _split_sync_waits(nc)
    return nc


def _split_sync_waits(nc, max_waits=1):
    """This walrus build accepts at most one sync-wait per instruction;
    move excess waits onto preceding same-engine NOPs."""
    import concourse.mybir as mybir

    uid = [0]
    for fn in nc.m.functions:
        for bb in fn.blocks:
            insts = bb.instructions
            out = []
            for inst in insts:
                si = inst.sync_info
                if si is not None and si.on_wait and len(si.on_wait) > max_waits:
                    waits = list(si.on_wait)
                    keep = waits[-max_waits:]
                    for w in waits[:-max_waits]:
                        uid[0] += 1
                        nop = mybir.InstNoOp(
                            name=f"I-waitsplit-{uid[0]}", ins=[], outs=[])
                        nop.engine = inst.engine
                        nop.sync_info = mybir.SyncInfo(on_wait=[w], on_update=[])
                        out.append(nop)
                    si.on_wait = keep
                out.append(inst)
            bb.instructions = out
    return nc


# ------------------------- host-side wrapper ----------------------------

def make_selectors(G6):
    """Host-built selector matrices for stats-row and broadcast matmuls."""
    selc = np.zeros((128, G6, G6), np.float32)
    for r in range(G6):
        selc[:, r, r] = 1.0
    selT = np.zeros((G6, G6, 128), np.float32)
    for r in range(G6):
        selT[r, r, :] = 1.0
    return (np.ascontiguousarray(selc.reshape(128, G6 * G6)),
            np.ascontiguousarray(selT.reshape(G6, G6 * 128)))


def _get_nc():
    key = "full"
    if key not in _NC_CACHE:
        _NC_CACHE[key] = build_nc()
    return _NC_CACHE[key]


def kernel(**inputs) -> np.ndarray:
    from concourse.bass_utils import run_bass_kernel_spmd

    votes = np.ascontiguousarray(np.asarray(inputs["votes"], dtype=np.float32))
    beta_v = np.asarray(inputs["beta_v"], dtype=np.float32).reshape(-1)
    beta_a = np.asarray(inputs["beta_a"], dtype=np.float32).reshape(-1)
    output_dim = int(np.asarray(inputs["output_dim"]))
    num_routing = int(np.asarray(inputs["num_routing"]))
    assert votes.shape == (B, I_DIM, C_DIM, D_DIM), votes.shape
    assert output_dim == O_DIM and num_routing == NUM_ROUTING

    CL = C_DIM // N_CORES
    G6, CG = CL // 24, 24
    bvc = np.repeat(beta_v, WW)   # [C]
    bac = np.repeat(beta_a, WW)
    nc = _get_nc()
    selc_np, selT_np = make_selectors(G6)
    in_maps = []
    for c in range(N_CORES):
        sl = slice(c * CL, (c + 1) * CL)
        in_maps.append({
            "votes": np.ascontiguousarray(votes[:, :, sl, :]),
            "bv16": np.ascontiguousarray(
                (P_DIM * bvc[sl]).reshape(G6, CG).astype(np.float32)),
            "ba": np.ascontiguousarray(bac[sl].reshape(G6, CG).astype(np.float32)),
            "selc": selc_np,
            "selT": selT_np,
        })
    res = run_bass_kernel_spmd(nc, in_maps, list(range(N_CORES)))
    parts = [res.results[i]["out"] for i in range(N_CORES)]
    full = np.concatenate(parts, axis=1)          # [B, C, D]
    w = int(math.sqrt(C_DIM // O_DIM))
    return full.reshape(B, O_DIM, w, w, D_DIM).astype(np.float32)


# revision 16
# speedup vs baseline: 1.3689x; 1.0148x over previous
"""Trainium2 Bass kernel for EM matrix-capsule routing (nn_MatrixRouting).

Problem shapes (hardcoded): votes [4, 1152, 1152, 17] f32, beta_v [1,32,1,1],
beta_a [1,32,1], output_dim=32, num_routing=3. Output [4, 32, 6, 6, 17].

Strategy: shard the output-capsule axis C=1152 across 8 cores (144 each).
All EM statistics (sums over the input-capsule axis I) are then core-local,
computed with selector-column matmuls on the TensorEngine (partition-dim
reduction). The only cross-core data needed is the R-normalization row-sum
(shape [B, I] = 18KB), all-reduced per batch with on-chip collectives.

Per EM iteration the kernel streams the 45MB/core votes shard from HBM:
  pass 0          : stats with R0 = const (folded out of mu/sigma).
  pass k (fused)  : phase-1 computes p/ap from iteration k-1 params and the
                    local row-sum partials; AllReduce; phase-2 re-streams the
                    shard to accumulate iteration-k stats with the new R.
Between passes, tiny [6, 384] "small math" computes mu/sigma/a and the
per-(c,p) exp-parameters, which are broadcast across partitions with K=6
selector-row matmuls.
"""

import math
import numpy as np
from contextlib import ExitStack

# ---- problem constants (hardcoded per the task contract) ----
B = 4
I_DIM = 1152
C_DIM = 1152
P_DIM = 16
D_DIM = 17
N_CORES = 8
NUM_ROUTING = 3
O_DIM = 32
WW = 36  # w*w = 6*6 positions per output capsule

EPS = 1e-10
LAMBDA = 1e-4
INV_SQRT_2PI = 1.0 / math.sqrt(2.0 * math.pi)

_NC_CACHE = {}


def _patch_tile_drain():
    """This walrus build only accepts one sync-wait on a CTRL instruction;
    spread the Tile exit-drain waits across single-wait NOPs."""
    import concourse.tile as tile
    import concourse.mybir as mybir
    from concourse.vector_clock import ScopedClock

    if getattr(tile.TileContext, "_drain_patched", False):
        return

    def _patched(self, tick_clock, wait_clock):
        nc = self.nc
        probe = nc.sync.nop()
        wait_clock.add_sem_waits(
            probe.ins, ScopedClock({None: tick_clock.global_clock})
        )
        waits = list(probe.ins.sync_info.on_wait) if probe.ins.sync_info else []
        if probe.ins.sync_info:
            probe.ins.sync_info.on_wait = waits[:1]
        for w in waits[1:]:
            n2 = nc.sync.nop()
            if n2.ins.sync_info is None:
                n2.ins.sync_info = mybir.SyncInfo(on_wait=[w], on_update=[])
            else:
                n2.ins.sync_info.on_wait = [w]
        nc.sync.drain()
        nc.all_engine_barrier()
        assert self.sems is not None
        popped = nc._tile_sem_poison_stack.pop()
        assert popped is self._sem_poison
        nc.clear_and_free_semaphores(list(self.sems.allocated().values()))
        nc.all_engine_barrier()

    tile.TileContext._drain_and_barrier = _patched
    tile.TileContext._drain_patched = True


def build_nc(Bd=B, Id=I_DIM, CL=C_DIM // N_CORES, n_cores=N_CORES,
             num_routing=NUM_ROUTING, Pd=P_DIM, split_waits=True):
    """Build the per-core SPMD Bass program (identical on every core)."""
    import concourse.bass as bass
    import concourse.mybir as mybir
    import concourse.tile as tile

    _patch_tile_drain()

    f32 = mybir.dt.float32
    bf = mybir.dt.bfloat16
    Dd = Pd + 1
    NCH = Id // 128          # i-chunks of 128 partitions
    assert Id % 128 == 0
    CG = 24                  # c's per stats row (CG*Pd = 384 <= 512 psum bank)
    G6 = CL // CG            # stats rows
    assert CL % CG == 0
    RW = CG * Pd             # 384 floats per stats row
    AX = mybir.AxisListType.X
    ALU = mybir.AluOpType
    ACTF = mybir.ActivationFunctionType

    nc = bass.Bass()
    votes = nc.declare_dram_parameter("votes", [Bd, Id, CL, Dd], f32, isOutput=False)
    bv16 = nc.declare_dram_parameter("bv16", [G6, CG], f32, isOutput=False)
    ba_in = nc.declare_dram_parameter("ba", [G6, CG], f32, isOutput=False)
    selc_in = nc.declare_dram_parameter("selc", [128, G6 * G6], f32, isOutput=False)
    selT_in = nc.declare_dram_parameter("selT", [G6, G6 * 128], f32, isOutput=False)
    out = nc.declare_dram_parameter("out", [Bd, CL, Dd], f32, isOutput=True)
    rs_loc = nc.dram_tensor("rs_loc", [Bd, Id], f32)
    rs_sh = nc.dram_tensor("rs_sh", [Bd, Id], f32, addr_space="Shared")
    votes_bv = nc.dram_tensor("votes_bv", [Bd, Id, CL, Pd], bf)
    votes_ba = nc.dram_tensor("votes_ba", [Bd, Id, CL], bf)

    groups = [list(range(n_cores))]

    with tile.TileContext(nc) as tc, ExitStack() as ctx:
        pconst = ctx.enter_context(tc.tile_pool(name="const", bufs=1))
        pv = ctx.enter_context(tc.tile_pool(name="vt", bufs=2))
        pw = ctx.enter_context(tc.tile_pool(name="work", bufs=1))
        pq = ctx.enter_context(tc.tile_pool(name="qs", bufs=2))
        pap = ctx.enter_context(tc.tile_pool(name="apb", bufs=2))
        prep = ctx.enter_context(tc.tile_pool(name="reps", bufs=4))
        psm = ctx.enter_context(tc.tile_pool(name="small", bufs=1))
        pps = ctx.enter_context(tc.tile_pool(name="psums", bufs=2, space="PSUM"))
        ppb = ctx.enter_context(tc.tile_pool(name="psumb", bufs=2, space="PSUM"))

        # -- constants (host-supplied selector matrices) --
        # selc[:, r, :] = [128, G6] with column r all-ones (stats row select)
        selc = pconst.tile([128, G6, G6], f32)
        nc.sync.dma_start(selc[:].rearrange("p a b -> p (a b)"), selc_in[:])
        # selT[:, r, :] = [G6, 128] with partition-row r all-ones (broadcast)
        selT = pconst.tile([G6, G6, 128], f32)
        nc.sync.dma_start(selT[:].rearrange("p a b -> p (a b)"), selT_in[:])
        bv16_t = pconst.tile([G6, CG], f32)
        nc.sync.dma_start(bv16_t[:], bv16[:])
        ba_t = pconst.tile([G6, CG], f32)
        nc.sync.dma_start(ba_t[:], ba_in[:])
        eps_col = pconst.tile([G6, 1], f32)
        nc.vector.memset(eps_col[:], EPS)
        selcb = pconst.tile([128, G6, G6], bf)
        nc.vector.tensor_copy(selcb[:], selc[:])
        selTb = pconst.tile([G6, G6, 128], bf)
        nc.vector.tensor_copy(selTb[:], selT[:])

        reps = {}  # b -> rep tiles from the previous EM iteration

        def stats_chunk(j, vtv, q_ap, ps0, ps1, ps2):
            """vtv: [128, CL, P] bf16; q_ap: [128, CL] bf16. Accumulate
            q, q*V, q*V^2 sums over i into f32 PSUM."""
            s1 = pq.tile([128, CL, Pd], bf, tag="s1")
            q_b = q_ap.unsqueeze(2).broadcast_to([128, CL, Pd])
            nc.vector.tensor_tensor(s1[:], vtv[:], q_b, op=ALU.mult)
            s2 = pq.tile([128, CL, Pd], bf, tag="s2")
            nc.vector.tensor_tensor(s2[:], s1[:], vtv[:], op=ALU.mult)
            s1f = s1[:].rearrange("p a b -> p (a b)")
            s2f = s2[:].rearrange("p a b -> p (a b)")
            for r in range(G6):
                st = selcb[:, r, :]
                first = j == 0 and r == 0
                last = j == NCH - 1 and r == G6 - 1
                nc.tensor.matmul(ps1[:], st, s1f[:, r * RW : (r + 1) * RW],
                                 start=first, stop=last)
                nc.tensor.matmul(ps2[:], st, s2f[:, r * RW : (r + 1) * RW],
                                 start=first, stop=last)
                nc.tensor.matmul(ps0[:], st, q_ap[:, r * CG : (r + 1) * CG],
                                 start=first, stop=last)

        for k in range(num_routing):
            for b in range(Bd):
                ps0 = pps.tile([G6, CG], f32, tag="ps0")
                ps1 = pps.tile([G6, RW], f32, tag="ps1")
                ps2 = pps.tile([G6, RW], f32, tag="ps2")

                if k == 0:
                    # stats with q = a_ (constant R0 = 1/O folded into sumR);
                    # also persist a bf16 copy of the shard for passes 1..n
                    for j in range(NCH):
                        vt = pv.tile([128, CL, Dd], f32, tag="vt")
                        nc.sync.dma_start(vt[:], votes[b, j * 128 : (j + 1) * 128])
                        vtv = pv.tile([128, CL, Pd], bf, tag="vtv")
                        nc.scalar.copy(vtv[:], vt[:, :, 0:Pd])
                        vta = pv.tile([128, CL], bf, tag="vta")
                        nc.vector.tensor_copy(vta[:], vt[:, :, Pd])
                        nc.sync.dma_start(votes_bv[b, j * 128 : (j + 1) * 128], vtv[:])
                        nc.sync.dma_start(votes_ba[b, j * 128 : (j + 1) * 128], vta[:])
                        stats_chunk(j, vtv, vta[:], ps0, ps1, ps2)
                else:
                    mu_rep, sa2_rep, e_rep = reps[b]
                    # ---- phase 1: p / ap / local rowsum ----
                    apb = pap.tile([128, NCH * CL], bf, tag="apb")
                    rs_all = pap.tile([128, NCH], f32, tag="rsall")
                    for j in range(NCH):
                        vtv = pv.tile([128, CL, Pd], bf, tag="vtv")
                        nc.sync.dma_start(vtv[:], votes_bv[b, j * 128 : (j + 1) * 128])
                        d = pw.tile([128, CL, Pd], bf, tag="d")
                        nc.vector.tensor_tensor(d[:], vtv[:], mu_rep[:],
                                                op=ALU.subtract)
                        nc.vector.tensor_tensor(d[:], d[:], sa2_rep[:], op=ALU.mult)
                        u = pw.tile([128, CL, Pd], bf, tag="u")
                        nc.scalar.activation(u[:], d[:], ACTF.Square)
                        nc.scalar.activation(u[:], u[:], ACTF.Exp, scale=-1.0)
                        w = pw.tile([128, CL, Pd], bf, tag="w")
                        nc.vector.tensor_tensor(w[:], u[:], e_rep[:], op=ALU.mult)
                        ap_sl = apb[:, j * CL : (j + 1) * CL]
                        with nc.allow_low_precision(
                                reason="16-term exp-sum; fp32 internal accum"):
                            nc.vector.reduce_sum(ap_sl, w[:], axis=AX)
                        nc.vector.reduce_sum(rs_all[:, j : j + 1], ap_sl, axis=AX)
                    nc.sync.dma_start(
                        rs_loc[b].rearrange("(j p) -> p j", p=128), rs_all[:]
                    )
                    nc.gpsimd.collective_compute(
                        "AllReduce", ALU.add, replica_groups=groups,
                        ins=[rs_loc[b]], outs=[rs_sh[b]],
                    )
                    rsg = pap.tile([128, NCH], f32, tag="rsg")
                    nc.sync.dma_start(
                        rsg[:], rs_sh[b].rearrange("(j p) -> p j", p=128)
                    )
                    nc.vector.tensor_scalar_add(rsg[:], rsg[:], EPS)
                    rcp = pap.tile([128, NCH], f32, tag="rcp")
                    nc.vector.reciprocal(rcp[:], rsg[:])
                    # ---- phase 2: stats with R_k = ap * rcp (times a_) ----
                    for j in range(NCH):
                        vtv = pv.tile([128, CL, Pd], bf, tag="vtv")
                        nc.sync.dma_start(vtv[:], votes_bv[b, j * 128 : (j + 1) * 128])
                        vta = pv.tile([128, CL], bf, tag="vta")
                        nc.sync.dma_start(vta[:], votes_ba[b, j * 128 : (j + 1) * 128])
                        ac = pq.tile([128, CL], f32, tag="ac")
                        nc.vector.tensor_copy(ac[:], vta[:])
                        apf = pq.tile([128, CL], f32, tag="apf")
                        nc.vector.tensor_copy(apf[:], apb[:, j * CL : (j + 1) * CL])
                        qf = pq.tile([128, CL], f32, tag="qf")
                        nc.vector.scalar_tensor_tensor(
                            qf[:], apf[:],
                            rcp[:, j : j + 1], ac[:],
                            op0=ALU.mult, op1=ALU.mult,
                        )
                        qb = pq.tile([128, CL], bf, tag="qb")
                        nc.vector.tensor_copy(qb[:], qf[:])
                        stats_chunk(j, vtv, qb[:], ps0, ps1, ps2)

                # ---- small math on [G6, RW]: mu, sigma, a, next-pass params --
                sb0 = psm.tile([G6, CG], f32, tag="sb0")
                nc.scalar.copy(sb0[:], ps0[:])
                sb1 = psm.tile([G6, CG, Pd], f32, tag="sb1")
                nc.scalar.copy(sb1[:].rearrange("p a b -> p (a b)"), ps1[:])
                sb2 = psm.tile([G6, CG, Pd], f32, tag="sb2")
                nc.scalar.copy(sb2[:].rearrange("p a b -> p (a b)"), ps2[:])

                rS = psm.tile([G6, CG], f32, tag="rS")
                nc.vector.reciprocal(rS[:], sb0[:])
                rS_b = rS[:].unsqueeze(2).broadcast_to([G6, CG, Pd])
                mu6 = psm.tile([G6, CG, Pd], f32, tag="mu6")
                nc.vector.tensor_tensor(mu6[:], sb1[:], rS_b, op=ALU.mult)
                ex2 = psm.tile([G6, CG, Pd], f32, tag="ex2")
                nc.vector.tensor_tensor(ex2[:], sb2[:], rS_b, op=ALU.mult)
                mu2 = psm.tile([G6, CG, Pd], f32, tag="mu2")
                nc.vector.tensor_tensor(mu2[:], mu6[:], mu6[:], op=ALU.mult)
                sig2 = psm.tile([G6, CG, Pd], f32, tag="sig2")
                nc.vector.tensor_tensor(sig2[:], ex2[:], mu2[:], op=ALU.subtract)
                sigma = psm.tile([G6, CG, Pd], f32, tag="sigma")
                nc.scalar.activation(sigma[:], sig2[:], ACTF.Sqrt)
                logs = psm.tile([G6, CG, Pd], f32, tag="logs")
                nc.scalar.activation(logs[:], sigma[:], ACTF.Ln, bias=eps_col[:])
                sumlog = psm.tile([G6, CG], f32, tag="sumlog")
                nc.vector.reduce_sum(sumlog[:], logs[:], axis=AX)
                cst = psm.tile([G6, CG], f32, tag="cst")
                nc.vector.tensor_tensor(cst[:], sumlog[:], bv16_t[:], op=ALU.add)
                if k == 0:
                    se = psm.tile([G6, CG], f32, tag="se")
                    nc.vector.tensor_scalar_mul(se[:], sb0[:], 1.0 / O_DIM)
                    sum_r_eff = se
                else:
                    sum_r_eff = sb0
                cst2 = psm.tile([G6, CG], f32, tag="cst2")
                nc.vector.tensor_tensor(cst2[:], cst[:], sum_r_eff[:], op=ALU.mult)
                ain = psm.tile([G6, CG], f32, tag="ain")
                nc.vector.tensor_tensor(ain[:], ba_t[:], cst2[:], op=ALU.subtract)
                a_t = psm.tile([G6, CG], f32, tag="a_t")
                nc.scalar.activation(a_t[:], ain[:], ACTF.Sigmoid, scale=LAMBDA)

                if k == num_routing - 1:
                    nc.sync.dma_start(
                        out[b, :, 0:Pd].rearrange("(r c) p -> r c p", r=G6), mu6[:]
                    )
                    nc.sync.dma_start(
                        out[b, :, Pd].rearrange("(r c) -> r c", r=G6), a_t[:]
                    )
                else:
                    # E = a * 1/(sigma+eps) * (2pi)^-1/2 ; sA2 = sqrt(1/(2 sig2))
                    sigeps = psm.tile([G6, CG, Pd], f32, tag="sigeps")
                    nc.vector.tensor_scalar_add(sigeps[:], sigma[:], EPS)
                    rsig = psm.tile([G6, CG, Pd], f32, tag="rsig")
                    nc.vector.reciprocal(rsig[:], sigeps[:])
                    a_b = a_t[:].unsqueeze(2).broadcast_to([G6, CG, Pd])
                    e6 = psm.tile([G6, CG, Pd], f32, tag="e6")
                    nc.vector.tensor_tensor(e6[:], rsig[:], a_b, op=ALU.mult)
                    nc.vector.tensor_scalar_mul(e6[:], e6[:], INV_SQRT_2PI)
                    t2s = psm.tile([G6, CG, Pd], f32, tag="t2s")
                    nc.vector.tensor_scalar_mul(t2s[:], sig2[:], 2.0)
                    rt2 = psm.tile([G6, CG, Pd], f32, tag="rt2")
                    nc.vector.reciprocal(rt2[:], t2s[:])
                    sa26 = psm.tile([G6, CG, Pd], f32, tag="sa26")
                    nc.scalar.activation(sa26[:], rt2[:], ACTF.Sqrt)

                    mu_rep = prep.tile([128, CL, Pd], bf, tag="mu_rep")
                    sa2_rep = prep.tile([128, CL, Pd], bf, tag="sa2_rep")
                    e_rep = prep.tile([128, CL, Pd], bf, tag="e_rep")
                    for (nm, src, rep) in (("mu", mu6, mu_rep),
                                           ("sa2", sa26, sa2_rep),
                                           ("e", e6, e_rep)):
                        srcb = psm.tile([G6, CG, Pd], bf, tag="srcb_" + nm)
                        nc.scalar.copy(srcb[:], src[:])
                        srcf = srcb[:].rearrange("p a b -> p (a b)")
                        repf = rep[:].rearrange("p a b -> p (a b)")
                        for r in range(G6):
                            pb = ppb.tile([128, RW], f32, tag="pb")
                            nc.tensor.matmul(pb[:], selTb[:, r, :],
                                             srcf[:, :], start=True, stop=True)
                            if r % 2 == 0:
                                nc.scalar.copy(repf[:, r * RW : (r + 1) * RW], pb[:])
                            else:
                                nc.vector.tensor_copy(
                                    repf[:, r * RW : (r + 1) * RW], pb[:])
                    reps[b] = (mu_rep, sa2_rep, e_rep)
    if split_waits:
        _split_sync_waits(nc)
    return nc


def _split_sync_waits(nc, max_waits=1):
    """This walrus build accepts at most one sync-wait per instruction;
    move excess waits onto preceding same-engine NOPs."""
    import concourse.mybir as mybir

    uid = [0]
    for fn in nc.m.functions:
        for bb in fn.blocks:
            insts = bb.instructions
            out = []
            for inst in insts:
                si = inst.sync_info
                if si is not None and si.on_wait and len(si.on_wait) > max_waits:
                    waits = list(si.on_wait)
                    keep = waits[-max_waits:]
                    for w in waits[:-max_waits]:
                        uid[0] += 1
                        nop = mybir.InstNoOp(
                            name=f"I-waitsplit-{uid[0]}", ins=[], outs=[])
                        nop.engine = inst.engine
                        nop.sync_info = mybir.SyncInfo(on_wait=[w], on_update=[])
                        out.append(nop)
                    si.on_wait = keep
                out.append(inst)
            bb.instructions = out
    return nc


# ------------------------- host-side wrapper ----------------------------

def make_selectors(G6):
    """Host-built selector matrices for stats-row and broadcast matmuls."""
    selc = np.zeros((128, G6, G6), np.float32)
    for r in range(G6):
        selc[:, r, r] = 1.0
    selT = np.zeros((G6, G6, 128), np.float32)
    for r in range(G6):
        selT[r, r, :] = 1.0
    return (np.ascontiguousarray(selc.reshape(128, G6 * G6)),
            np.ascontiguousarray(selT.reshape(G6, G6 * 128)))


def _get_nc():
    key = "full"
    if key not in _NC_CACHE:
        _NC_CACHE[key] = build_nc()
    return _NC_CACHE[key]


def kernel(**inputs) -> np.ndarray:
    from concourse.bass_utils import run_bass_kernel_spmd

    votes = np.ascontiguousarray(np.asarray(inputs["votes"], dtype=np.float32))
    beta_v = np.asarray(inputs["beta_v"], dtype=np.float32).reshape(-1)
    beta_a = np.asarray(inputs["beta_a"], dtype=np.float32).reshape(-1)
    output_dim = int(np.asarray(inputs["output_dim"]))
    num_routing = int(np.asarray(inputs["num_routing"]))
    assert votes.shape == (B, I_DIM, C_DIM, D_DIM), votes.shape
    assert output_dim == O_DIM and num_routing == NUM_ROUTING

    CL = C_DIM // N_CORES
    G6, CG = CL // 24, 24
    bvc = np.repeat(beta_v, WW)   # [C]
    bac = np.repeat(beta_a, WW)
    nc = _get_nc()
    selc_np, selT_np = make_selectors(G6)
    in_maps = []
    for c in range(N_CORES):
        sl = slice(c * CL, (c + 1) * CL)
        in_maps.append({
            "votes": np.ascontiguousarray(votes[:, :, sl, :]),
            "bv16": np.ascontiguousarray(
                (P_DIM * bvc[sl]).reshape(G6, CG).astype(np.float32)),
            "ba": np.ascontiguousarray(bac[sl].reshape(G6, CG).astype(np.float32)),
            "selc": selc_np,
            "selT": selT_np,
        })
    res = run_bass_kernel_spmd(nc, in_maps, list(range(N_CORES)))
    parts = [res.results[i]["out"] for i in range(N_CORES)]
    full = np.concatenate(parts, axis=1)          # [B, C, D]
    w = int(math.sqrt(C_DIM // O_DIM))
    return full.reshape(B, O_DIM, w, w, D_DIM).astype(np.float32)
